# revision 1
# baseline (speedup 1.0000x reference)
"""Trainium2 Bass kernel for a pre-LN transformer encoder layer.

Model: D_MODEL=1024, N_HEADS=16, D_K=64, D_FF=4096, B=2, S=2048, fp32 I/O.

Sharding: fully data-parallel over 8 cores = (batch b, query-block j) with
512 query tokens per core.  Each core recomputes LN1/K/V for its full batch
element (no collectives), computes attention + FFN for its own 512 tokens,
and writes its [512, 1024] slice of the output.  Per-core inputs are rotated
so the core's own query block is always tokens [0:512) (attention is exactly
permutation-invariant over keys, so rotating keys+mask together is safe).

Layout strategy on device: activations are kept feature-major ("transposed",
features on partitions) so every projection consumes the previous output
directly; softmax runs on transposed scores S^T[k, q] with the mask folded
into the Exp bias (per-partition) and the row-sum obtained via an appended
ones-column on V.  Matmuls are bf16 with fp32 PSUM accumulation.

Host-side preprocessing (exact linear-algebra folds):
  - LN1 affine folded into wq/wk/wv;  1/sqrt(d_k) folded into wq
  - V bias folded into the O-projection bias (softmax rows sum to 1)
  - LN2 affine folded into w1
"""

import sys

sys.path.insert(0, "/opt/trn_rl_repo")

import numpy as np
import ml_dtypes

import concourse.bass as bass
import concourse.tile as tile
from concourse import bacc, mybir
from concourse.bass_utils import run_bass_kernel_spmd
from concourse.masks import make_identity

F32 = mybir.dt.float32
BF16 = mybir.dt.bfloat16
AF = mybir.ActivationFunctionType
ALU = mybir.AluOpType

D = 1024          # d_model
H = 16            # heads
DK = 64           # head dim
DFF = 4096        # ffn hidden
S = 2048          # keys per batch element (per core)
Q = 512           # query tokens per core
EPS = 1e-6
NCHIP = 8
VS = 68           # V slot stride per head (64 data + ones col + 3 pad, 4B aligned)


def _build():
    nc = bacc.Bacc("TRN2", target_bir_lowering=False, debug=False)

    x_all = nc.dram_tensor("x_all", [S, D], F32, kind="ExternalInput").ap()
    wq_d = nc.dram_tensor("wq_b", [D, D], BF16, kind="ExternalInput").ap()
    wk_d = nc.dram_tensor("wk_b", [D, D], BF16, kind="ExternalInput").ap()
    wv_d = nc.dram_tensor("wv_b", [D, D], BF16, kind="ExternalInput").ap()
    wo_d = nc.dram_tensor("wo_b", [D, D], BF16, kind="ExternalInput").ap()
    w1_d = nc.dram_tensor("w1_b", [D, DFF], BF16, kind="ExternalInput").ap()
    w2_d = nc.dram_tensor("w2_b", [DFF, D], BF16, kind="ExternalInput").ap()
    bq_d = nc.dram_tensor("bq_v", [D], F32, kind="ExternalInput").ap()
    bk_d = nc.dram_tensor("bk_v", [D], F32, kind="ExternalInput").ap()
    bo_d = nc.dram_tensor("bo_v", [D], F32, kind="ExternalInput").ap()
    b1_d = nc.dram_tensor("b1_v", [DFF], F32, kind="ExternalInput").ap()
    b2_d = nc.dram_tensor("b2_v", [D], F32, kind="ExternalInput").ap()
    mb_d = nc.dram_tensor("mb_v", [S], F32, kind="ExternalInput").ap()
    out_d = nc.dram_tensor("out", [Q, D], F32, kind="ExternalOutput").ap()

    with tile.TileContext(nc) as tc:
        _emit(nc, tc, x_all, wq_d, wk_d, wv_d, wo_d, w1_d, w2_d,
              bq_d, bk_d, bo_d, b1_d, b2_d, mb_d, out_d)
    nc.compile()
    return nc


def _emit(nc, tc, *args):
    import os
    R = int(os.environ.get("BASS_REPEAT", "1"))
    if R > 1:
        with tc.For_i(0, R, 1):
            _emit_body(nc, tc, *args)
    else:
        _emit_body(nc, tc, *args)


def _emit_body(nc, tc, x_all, wq_d, wk_d, wv_d, wo_d, w1_d, w2_d,
               bq_d, bk_d, bo_d, b1_d, b2_d, mb_d, out_d):
    import os
    PHASES = os.environ.get("BASS_PHASES", "ABCDE")

    def _dummy_out():
        # debug truncation: satisfy the output with a DRAM->DRAM copy
        nc.sync.dma_start(out_d[:, :], x_all[0:Q, :])

    NT = S // 128          # 16 token tiles of the full batch element
    NQ = Q // 128          # 4 token tiles of own block
    NR = D // 128          # 8 feature chunks of d_model
    NF = DFF // 128        # 32 feature chunks of d_ff

    consts = tc.alloc_tile_pool(name="consts", bufs=1)
    ident_bf = consts.tile([128, 128], BF16)
    make_identity(nc, ident_bf)
    ident_f = consts.tile([128, 128], F32)
    make_identity(nc, ident_f)
    bq_sb = consts.tile([128, NR], F32)
    nc.sync.dma_start(bq_sb[:], bq_d.rearrange("(m p) -> p m", p=128))
    bk_sb = consts.tile([128, NR], F32)
    nc.sync.dma_start(bk_sb[:], bk_d.rearrange("(m p) -> p m", p=128))
    bo_sb = consts.tile([128, NR], F32)
    nc.sync.dma_start(bo_sb[:], bo_d.rearrange("(m p) -> p m", p=128))
    b1_sb = consts.tile([128, NF], F32)
    nc.sync.dma_start(b1_sb[:], b1_d.rearrange("(f p) -> p f", p=128))
    mb_sb = consts.tile([128, NT], F32)
    nc.sync.dma_start(mb_sb[:], mb_d.rearrange("(i p) -> p i", p=128))

    psum = tc.alloc_tile_pool(name="psum", bufs=1, space="PSUM")
    dram = tc.alloc_tile_pool(name="dram", bufs=1, space="DRAM")

    # ---- weight pools (right-side stack: poolA, poolWv, poolWqk) ------
    poolA = tc.alloc_tile_pool(name="poolA", bufs=1, side="right")  # ln1T, lives into C
    ln1T = poolA.tile([128, NR * S], BF16)               # chunk r at [r*S, +S)
    poolWv = tc.alloc_tile_pool(name="poolWv", bufs=1, side="right")  # wv, lives into C
    wv_sb = poolWv.tile([128, NR * D], BF16)
    poolWqk = tc.alloc_tile_pool(name="poolWqk", bufs=1, side="right")  # wq/wk, phase B
    wq_sb = poolWqk.tile([128, NR * D], BF16)
    wk_sb = poolWqk.tile([128, NR * D], BF16)
    # NOTE: the weight DMAs are emitted AFTER phase A's loop so the x tiles
    # (which gate all PE work) aren't queued behind 24 MiB of weights

    # ---- phase A: LN1 over all S tokens, transpose to feature-major ---
    streamA = tc.alloc_tile_pool(name="streamA", bufs=3, side="right")
    for t in range(NT):
        xt = streamA.tile([128, D], F32, bufs=3)
        nc.sync.dma_start(xt[:], x_all[t * 128:(t + 1) * 128, :])
        stats = streamA.tile([128, 2, 6], F32, bufs=4)
        xg = xt.rearrange("p (g d) -> p g d", g=2)
        nc.vector.bn_stats(stats[:, 0, :], xg[:, 0, :])
        nc.vector.bn_stats(stats[:, 1, :], xg[:, 1, :])
        mv = streamA.tile([128, 2], F32, bufs=4)
        nc.vector.bn_aggr(mv[:], stats[:])
        rstd = streamA.tile([128, 1], F32, bufs=4)
        # std with Bessel correction (ddof=1), then 1/(std+eps)
        nc.scalar.activation(rstd[:], mv[:, 1:2], AF.Sqrt, scale=float(D) / (D - 1))
        nc.vector.tensor_scalar_add(rstd[:], rstd[:], EPS)
        nc.vector.reciprocal(rstd[:], rstd[:])
        lt = streamA.tile([128, D], BF16, bufs=3)
        nc.vector.tensor_scalar(
            out=lt[:], in0=xt[:], scalar1=mv[:, 0:1], scalar2=rstd[:],
            op0=ALU.subtract, op1=ALU.mult)
        for r in range(NR):
            tp = psum.tile([128, 128], BF16, tag="tr", bufs=2)
            nc.tensor.transpose(tp[:], lt[:, r * 128:(r + 1) * 128], ident_bf[:])
            nc.vector.tensor_copy(ln1T[:, r * S + t * 128: r * S + (t + 1) * 128], tp[:])
    for r in range(NR):
        nc.sync.dma_start(wq_sb[:, r * D:(r + 1) * D], wq_d[r * 128:(r + 1) * 128, :])
        nc.sync.dma_start(wk_sb[:, r * D:(r + 1) * D], wk_d[r * 128:(r + 1) * 128, :])
        nc.sync.dma_start(wv_sb[:, r * D:(r + 1) * D], wv_d[r * 128:(r + 1) * 128, :])
    streamA.release()
    if "B" not in PHASES:
        _dummy_out()
        poolWqk.release(); poolWv.release(); poolA.release()
        dram.release(); consts.release(); psum.release()
        return

    # ---- phase B: Q^T, K^T (feature-major) and V (token-major) --------
    poolQKV = tc.alloc_tile_pool(name="poolQKV", bufs=1)
    QT = poolQKV.tile([128, NR * Q], BF16)        # chunk m at [m*Q, +Q)
    KT = poolQKV.tile([128, NR * S], BF16)        # chunk m at [m*S, +S)
    V_sb = poolQKV.tile([128, NT * 16 * VS], BF16)  # tok chunk t at [t*16*VS), head h at +h*VS
    Ou = poolQKV.tile([128, NR * Q], BF16)        # normalized attn out, QT layout
    # per-head softmax sums staged at 32-aligned partitions (DVE base-partition
    # rule), head h at (partition 32*(h//4), column block h%4)
    sums_st = poolQKV.tile([128, 4 * Q], F32)
    sums_all = poolQKV.tile([16, Q], F32)
    recipb = poolQKV.tile([16, Q], BF16)
    recip = poolQKV.tile([16, Q], F32)
    Vv = V_sb.rearrange("p (t h s) -> p t h s", t=NT, s=VS)
    nc.vector.memset(Vv[:, :, :, 64:65], 1.0)     # ones column for row-sums

    for m in range(NR):
        qps = psum.tile([128, Q], F32, tag="mm512", bufs=4)
        for r in range(NR):
            nc.tensor.matmul(
                qps[:], wq_sb[:, r * D + m * 128: r * D + (m + 1) * 128],
                ln1T[:, r * S: r * S + Q], start=(r == 0), stop=(r == NR - 1))
        nc.vector.tensor_scalar_add(QT[:, m * Q:(m + 1) * Q], qps[:], bq_sb[:, m:m + 1])
    for m in range(NR):
        for s4 in range(S // Q):
            kps = psum.tile([128, Q], F32, tag="mm512", bufs=4)
            for r in range(NR):
                nc.tensor.matmul(
                    kps[:], wk_sb[:, r * D + m * 128: r * D + (m + 1) * 128],
                    ln1T[:, r * S + s4 * Q: r * S + (s4 + 1) * Q],
                    start=(r == 0), stop=(r == NR - 1))
            nc.vector.tensor_scalar_add(
                KT[:, m * S + s4 * Q: m * S + (s4 + 1) * Q], kps[:], bk_sb[:, m:m + 1])
    def _v_proj_group(t, s2):
        vps = psum.tile([128, Q], F32, tag="mm512", bufs=4, name="vps")
        for r in range(NR):
            nc.tensor.matmul(
                vps[:], ln1T[:, r * S + t * 128: r * S + (t + 1) * 128],
                wv_sb[:, r * D + s2 * Q: r * D + (s2 + 1) * Q],
                start=(r == 0), stop=(r == NR - 1))
        nc.vector.tensor_copy(
            Vv[:, t, s2 * 8:(s2 + 1) * 8, 0:64],
            vps.rearrange("p (h d) -> p h d", d=64))

    # V heads 0-7 (s2=0) now; heads 8-15 (s2=1) are emitted inside the
    # attention pair loop to fill the PE slack while ACT runs the exps
    for t in range(NT):
        _v_proj_group(t, 0)
    poolWqk.release()
    if "C" not in PHASES:
        _dummy_out()
        poolWv.release(); poolA.release()
        poolQKV.release()
        dram.release(); consts.release(); psum.release()
        return

    # ---- phase C: attention per head ---------------------------------
    poolE = tc.alloc_tile_pool(name="poolE", bufs=1)
    E0 = poolE.tile([128, NT * Q], BF16)
    E1 = poolE.tile([128, NT * Q], BF16)

    # heads processed in pairs (2m, 2m+1): their K=64 score matmuls target
    # disjoint PE row groups (base partitions 0 / 64) and run concurrently
    for m in range(NR):
        for i in range(NT):
            spsA = psum.tile([128, Q], F32, tag="mm512", bufs=4)
            spsB = psum.tile([128, Q], F32, tag="mm512", bufs=4)
            ks = m * S + i * 128
            nc.tensor.matmul(
                spsA[:], KT[0:64, ks:ks + 128],
                QT[0:64, m * Q:(m + 1) * Q], start=True, stop=True)
            nc.tensor.matmul(
                spsB[:], KT[64:128, ks:ks + 128],
                QT[64:128, m * Q:(m + 1) * Q], start=True, stop=True)
            nc.scalar.activation(
                E0[:, i * Q:(i + 1) * Q], spsA[:], AF.Exp,
                bias=mb_sb[:, i:i + 1], scale=1.0)
            nc.scalar.activation(
                E1[:, i * Q:(i + 1) * Q], spsB[:], AF.Exp,
                bias=mb_sb[:, i:i + 1], scale=1.0)
        if m < 4:   # fill PE slack under the exp stream with V heads 8-15
            for t in range(4 * m, 4 * m + 4):
                _v_proj_group(t, 1)
        opsA = psum.tile([128, Q], F32, tag="av", bufs=2)
        opsB = psum.tile([128, Q], F32, tag="av", bufs=2)
        for i in range(NT):
            vs = i * 16 * VS
            nc.tensor.matmul(
                opsA[0:65, :], V_sb[:, vs + 2 * m * VS: vs + 2 * m * VS + 65],
                E0[:, i * Q:(i + 1) * Q], start=(i == 0), stop=(i == NT - 1),
                skip_group_check=True)
            nc.tensor.matmul(
                opsB[0:65, :], V_sb[:, vs + (2 * m + 1) * VS: vs + (2 * m + 1) * VS + 65],
                E1[:, i * Q:(i + 1) * Q], start=(i == 0), stop=(i == NT - 1),
                skip_group_check=True)
        for h, ops in ((2 * m, opsA), (2 * m + 1, opsB)):
            half = (h % 2) * 64
            p4, c4 = 32 * (h // 4), (h % 4) * Q
            nc.vector.tensor_copy(sums_st[p4:p4 + 1, c4:c4 + Q], ops[64:65, :])
            nc.vector.tensor_copy(Ou[half:half + 64, m * Q:(m + 1) * Q], ops[0:64, :])

    poolE.release()
    poolWv.release()
    poolA.release()

    poolD1 = tc.alloc_tile_pool(name="poolD1", bufs=1, side="right")  # x_own, mhaT, g
    x_own = poolD1.tile([128, NQ * D], F32)
    for t in range(NQ):
        nc.sync.dma_start(x_own[:, t * D:(t + 1) * D], x_all[t * 128:(t + 1) * 128, :])
    poolwo = tc.alloc_tile_pool(name="poolwo", bufs=1, side="right")
    wo_sb = poolwo.tile([128, NR * D], BF16)
    for r in range(NR):
        nc.sync.dma_start(wo_sb[:, r * D:(r + 1) * D], wo_d[r * 128:(r + 1) * 128, :])

    sums_src = sums_st.rearrange("(a b) (h q) -> a b h q", b=32, h=4)[:, 0, :, :]
    nc.sync.dma_start(sums_all[:, :], sums_src)
    nc.vector.reciprocal(recip[:], sums_all[:])
    nc.vector.tensor_copy(recipb[:], recip[:])
    rb_d = dram.tile([16, Q], BF16)
    nc.sync.dma_start(rb_d[:], recipb[:])
    streamC = tc.alloc_tile_pool(name="streamC", bufs=2)
    for m in range(NR):     # head pair (2m, 2m+1) shares partition chunk m
        rb = streamC.tile([128, Q], BF16, bufs=2)
        nc.sync.dma_start(rb[0:64, :], rb_d[2 * m:2 * m + 1, :].to_broadcast([64, Q]))
        nc.sync.dma_start(rb[64:128, :], rb_d[2 * m + 1:2 * m + 2, :].to_broadcast([64, Q]))
        nc.vector.tensor_tensor(
            out=Ou[:, m * Q:(m + 1) * Q],
            in0=Ou[:, m * Q:(m + 1) * Q], in1=rb[:], op=ALU.mult)
    streamC.release()
    if "D" not in PHASES:
        _dummy_out()
        poolwo.release(); poolD1.release(); poolQKV.release()
        dram.release(); consts.release(); psum.release()
        return

    # ---- phase D: O-projection, residual, LN2, transpose g -----------
    mhaT = poolD1.tile([128, NR * Q], F32)
    for m in range(NR):
        mps = psum.tile([128, Q], F32, tag="mm512", bufs=4)
        for r in range(NR):
            nc.tensor.matmul(
                mps[:], wo_sb[:, r * D + m * 128: r * D + (m + 1) * 128],
                Ou[:, r * Q:(r + 1) * Q], start=(r == 0), stop=(r == NR - 1))
        nc.vector.tensor_scalar_add(mhaT[:, m * Q:(m + 1) * Q], mps[:], bo_sb[:, m:m + 1])
    poolwo.release()
    poolQKV.release()

    poolD2 = tc.alloc_tile_pool(name="poolD2", bufs=1)   # h, gT, b2b (live into E)
    h_sb = poolD2.tile([128, NQ * D], F32)
    gT = poolD2.tile([128, NR * Q], BF16)                # chunk r at [r*Q, +Q)
    b2b = poolD2.tile([128, D], F32)
    b2_bc = bass.AP(tensor=b2_d.tensor, offset=b2_d.offset, ap=[[0, 128]] + list(b2_d.ap))
    nc.sync.dma_start(b2b[:], b2_bc)

    for t in range(NQ):
        for r in range(NR):
            tp = psum.tile([128, 128], F32, tag="tr", bufs=2)
            nc.tensor.transpose(
                tp[:], mhaT[:, r * Q + t * 128: r * Q + (t + 1) * 128], ident_f[:])
            nc.vector.tensor_tensor(
                out=h_sb[:, t * D + r * 128: t * D + (r + 1) * 128],
                in0=tp[:], in1=x_own[:, t * D + r * 128: t * D + (r + 1) * 128],
                op=ALU.add)
    g_sb = poolD1.tile([128, NQ * D], BF16)
    streamD = tc.alloc_tile_pool(name="streamD", bufs=4, side="right")
    for t in range(NQ):
        ht = h_sb[:, t * D:(t + 1) * D]
        stats2 = streamD.tile([128, 2, 6], F32, bufs=4)
        hg = ht.rearrange("p (g d) -> p g d", g=2)
        nc.vector.bn_stats(stats2[:, 0, :], hg[:, 0, :])
        nc.vector.bn_stats(stats2[:, 1, :], hg[:, 1, :])
        mv2 = streamD.tile([128, 2], F32, bufs=4)
        nc.vector.bn_aggr(mv2[:], stats2[:])
        rstd2 = streamD.tile([128, 1], F32, bufs=4)
        nc.scalar.activation(rstd2[:], mv2[:, 1:2], AF.Sqrt, scale=float(D) / (D - 1))
        nc.vector.tensor_scalar_add(rstd2[:], rstd2[:], EPS)
        nc.vector.reciprocal(rstd2[:], rstd2[:])
        nc.vector.tensor_scalar(
            out=g_sb[:, t * D:(t + 1) * D], in0=ht, scalar1=mv2[:, 0:1],
            scalar2=rstd2[:], op0=ALU.subtract, op1=ALU.mult)
    streamD.release()
    for t in range(NQ):
        for r in range(NR):
            tg = psum.tile([128, 128], BF16, tag="tr", bufs=2)
            nc.tensor.transpose(
                tg[:], g_sb[:, t * D + r * 128: t * D + (r + 1) * 128], ident_bf[:])
            nc.vector.tensor_copy(gT[:, r * Q + t * 128: r * Q + (t + 1) * 128], tg[:])
    # h2 = h + b2 (after g extracted)
    for t in range(NQ):
        nc.vector.tensor_tensor(
            out=h_sb[:, t * D:(t + 1) * D], in0=h_sb[:, t * D:(t + 1) * D],
            in1=b2b[:], op=ALU.add)
    poolD1.release()
    if "E" not in PHASES:
        _dummy_out()
        poolD2.release()
        dram.release(); consts.release(); psum.release()
        return

    # ---- phase E: FFN -------------------------------------------------
    poolF = tc.alloc_tile_pool(name="poolF", bufs=1)
    w1_sb = poolF.tile([128, NR * DFF], BF16)     # din chunk r at [r*DFF, +DFF)
    for r in range(NR):
        nc.sync.dma_start(w1_sb[:, r * DFF:(r + 1) * DFF], w1_d[r * 128:(r + 1) * 128, :])
    w2_sb = poolF.tile([128, NF * D], BF16)       # dff chunk f at [f*D, +D)
    for f in range(NF):
        nc.sync.dma_start(w2_sb[:, f * D:(f + 1) * D], w2_d[f * 128:(f + 1) * 128, :])
    H1T = poolF.tile([128, NF * Q], BF16)         # dff chunk f at [f*Q, +Q)

    for f in range(NF):
        fps = psum.tile([128, Q], F32, tag="mm512", bufs=4)
        for r in range(NR):
            nc.tensor.matmul(
                fps[:], w1_sb[:, r * DFF + f * 128: r * DFF + (f + 1) * 128],
                gT[:, r * Q:(r + 1) * Q], start=(r == 0), stop=(r == NR - 1))
        nc.scalar.activation(
            H1T[:, f * Q:(f + 1) * Q], fps[:], AF.Gelu, bias=b1_sb[:, f:f + 1], scale=1.0)

    streamE = tc.alloc_tile_pool(name="streamE", bufs=2)
    for t in range(NQ):
        for s2 in range(2):
            ops2 = psum.tile([128, Q], F32, tag="av", bufs=2)
            for f in range(NF):
                nc.tensor.matmul(
                    ops2[:], H1T[:, f * Q + t * 128: f * Q + (t + 1) * 128],
                    w2_sb[:, f * D + s2 * Q: f * D + (s2 + 1) * Q],
                    start=(f == 0), stop=(f == NF - 1))
            ot = streamE.tile([128, Q], F32, bufs=2)
            nc.vector.tensor_tensor(
                out=ot[:], in0=ops2[:],
                in1=h_sb[:, t * D + s2 * Q: t * D + (s2 + 1) * Q], op=ALU.add)
            nc.sync.dma_start(out_d[t * 128:(t + 1) * 128, s2 * Q:(s2 + 1) * Q], ot[:])
    streamE.release()
    poolF.release()
    poolD2.release()
    dram.release()
    consts.release()
    psum.release()


_NC = None


def _get_nc():
    global _NC
    if _NC is None:
        _NC = _build()
    return _NC


def _prep_in_maps(inputs):
    x = np.asarray(inputs["x"], np.float32)          # [2, 2048, 1024]
    mask = np.asarray(inputs["mask"])                # [2, 1, 1, 2048]
    wq, bq = np.asarray(inputs["wq"], np.float32), np.asarray(inputs["bq"], np.float32)
    wk, bk = np.asarray(inputs["wk"], np.float32), np.asarray(inputs["bk"], np.float32)
    wv, bv = np.asarray(inputs["wv"], np.float32), np.asarray(inputs["bv"], np.float32)
    wo, bo = np.asarray(inputs["wo"], np.float32), np.asarray(inputs["bo"], np.float32)
    ln1_w, ln1_b = np.asarray(inputs["ln1_w"], np.float32), np.asarray(inputs["ln1_b"], np.float32)
    ln2_w, ln2_b = np.asarray(inputs["ln2_w"], np.float32), np.asarray(inputs["ln2_b"], np.float32)
    w1, b1 = np.asarray(inputs["w1"], np.float32), np.asarray(inputs["b1"], np.float32)
    w2, b2 = np.asarray(inputs["w2"], np.float32), np.asarray(inputs["b2"], np.float32)

    bf = ml_dtypes.bfloat16
    sc = 1.0 / np.sqrt(np.float32(DK))
    wq_b = (ln1_w[:, None] * wq * sc).astype(bf)
    wk_b = (ln1_w[:, None] * wk).astype(bf)
    wv_b = (ln1_w[:, None] * wv).astype(bf)
    wo_b = wo.astype(bf)
    w1_b = (ln2_w[:, None] * w1).astype(bf)
    w2_b = w2.astype(bf)
    bq_v = ((ln1_b @ wq + bq) * sc).astype(np.float32)
    bk_v = (ln1_b @ wk + bk).astype(np.float32)
    bv_full = ln1_b @ wv + bv
    bo_v = (bv_full @ wo + bo).astype(np.float32)
    b1_v = (ln2_b @ w1 + b1).astype(np.float32)
    b2_v = b2.astype(np.float32)

    common = dict(wq_b=wq_b, wk_b=wk_b, wv_b=wv_b, wo_b=wo_b, w1_b=w1_b,
                  w2_b=w2_b, bq_v=bq_v, bk_v=bk_v, bo_v=bo_v, b1_v=b1_v,
                  b2_v=b2_v)
    in_maps = []
    for c in range(NCHIP):
        b, j = c // 4, c % 4
        q0 = j * Q
        xr = np.concatenate([x[b, q0:], x[b, :q0]], axis=0)
        mbias = np.where(mask[b, 0, 0] == 0, np.float32(-30000.0), np.float32(0.0))
        mbr = np.concatenate([mbias[q0:], mbias[:q0]]).astype(np.float32)
        in_maps.append(dict(common, x_all=np.ascontiguousarray(xr), mb_v=mbr))
    return in_maps


def kernel(**inputs):
    in_maps = _prep_in_maps(inputs)
    nc = _get_nc()
    res = run_bass_kernel_spmd(nc, in_maps, core_ids=list(range(NCHIP)))
    out = np.empty((2, S, D), np.float32)
    for c in range(NCHIP):
        b, j = c // 4, c % 4
        out[b, j * Q:(j + 1) * Q] = res.results[c]["out"]
    return out



# revision 2
# speedup vs baseline: 1.6283x; 1.6283x over previous
"""Trainium2 Bass kernel for a pre-LN transformer encoder layer (v2).

Model: D_MODEL=1024, N_HEADS=16, D_K=64, D_FF=4096, B=2, S=2048, fp32 I/O.

Sharding: fully data-parallel over 8 cores = (batch b, query-block j) with
512 query tokens per core.  Each core recomputes LN1/K/V for its full batch
element (no collectives), computes attention + FFN for its own 512 tokens,
and writes its [512, 1024] slice of the output.  Per-core inputs are rotated
so the core's own query block is always tokens [0:512).

v2 changes vs v1 (677us baseline):
  - all weight/x DMAs prefetched into phase slack (wo/x_own at C start,
    w1 at C end, w2 mid-D) instead of just-in-time
  - softmax exps fused to 1024-wide (2 key-tiles per ACT instruction)
  - softmax normalization via PE rank-1 broadcast of 1/rowsum (no DRAM
    round-trip, pipelined per head pair inside phase C)
  - phase A/D LayerNorm application moved to ACT (scale/bias APs),
    transpose PSUM->SBUF copies batched 4 tiles per instruction
  - K/V projections accumulate in 1024-wide PSUM tile pairs (fewer DVE ops)
"""

import os
import sys

sys.path.insert(0, "/opt/trn_rl_repo")

import numpy as np
import ml_dtypes

import concourse.bass as bass
import concourse.tile as tile
from concourse import bacc, mybir
from concourse.bass_utils import run_bass_kernel_spmd
from concourse.masks import make_identity

F32 = mybir.dt.float32
BF16 = mybir.dt.bfloat16
AF = mybir.ActivationFunctionType
ALU = mybir.AluOpType

D = 1024          # d_model
H = 16            # heads
DK = 64           # head dim
DFF = 4096        # ffn hidden
S = 2048          # keys per batch element (per core)
Q = 512           # query tokens per core
EPS = 1e-6
NCHIP = 8
VS = 68           # V slot stride per head (64 data + ones col + 3 pad)

NT = S // 128     # 16 key tiles
NQ = Q // 128     # 4 own-token tiles
NR = D // 128     # 8 feature chunks of d_model
NF = DFF // 128   # 32 feature chunks of d_ff


def _build(has_mask=False):
    nc = bacc.Bacc("TRN2", target_bir_lowering=False, debug=False)

    t_in = {}
    t_in["x_all"] = nc.dram_tensor("x_all", [S, D], F32, kind="ExternalInput").ap()
    for nm, shp in (("wq_b", [D, D]), ("wk_b", [D, D]), ("wv_b", [D, D]),
                    ("wo_b", [D, D]), ("w1_b", [D, DFF]), ("w2_b", [DFF, D])):
        t_in[nm] = nc.dram_tensor(nm, shp, BF16, kind="ExternalInput").ap()
    for nm, shp in (("bq_v", [D]), ("bk_v", [D]), ("bo_v", [D]),
                    ("b1_v", [DFF]), ("b2_v", [D])):
        t_in[nm] = nc.dram_tensor(nm, shp, F32, kind="ExternalInput").ap()
    if has_mask:
        t_in["mb_v"] = nc.dram_tensor("mb_v", [S], F32, kind="ExternalInput").ap()
    out_d = nc.dram_tensor("out", [Q, D], F32, kind="ExternalOutput").ap()

    with tile.TileContext(nc) as tc:
        R = int(os.environ.get("BASS_REPEAT", "1"))
        if R > 1:
            with tc.For_i(0, R, 1):
                _emit_body(nc, tc, t_in, out_d, has_mask)
        else:
            _emit_body(nc, tc, t_in, out_d, has_mask)
    nc.compile()
    return nc


def _emit_body(nc, tc, t_in, out_d, has_mask):
    PHASES = os.environ.get("BASS_PHASES", "ABCDE")
    x_all = t_in["x_all"]

    def _dummy_out():
        nc.sync.dma_start(out_d[:, :], x_all[0:Q, :])

    # ---------------- consts ----------------
    consts = tc.alloc_tile_pool(name="consts", bufs=1)
    ident_bf = consts.tile([128, 128], BF16)
    make_identity(nc, ident_bf)
    ident_f = consts.tile([128, 128], F32)
    make_identity(nc, ident_f)
    # bc selector: out[0:64] <- rec[row0], out[64:128] <- rec[row64]
    ones2 = consts.tile([128, 128], F32)
    nc.vector.memset(ones2[:], 0.0)
    nc.vector.memset(ones2[0:1, 0:64], 1.0)
    nc.vector.memset(ones2[64:65, 64:128], 1.0)
    bq_sb = consts.tile([128, NR], F32)
    nc.sync.dma_start(bq_sb[:], t_in["bq_v"].rearrange("(m p) -> p m", p=128))
    bk_sb = consts.tile([128, NR], F32)
    nc.sync.dma_start(bk_sb[:], t_in["bk_v"].rearrange("(m p) -> p m", p=128))
    bo_sb = consts.tile([128, NR], F32)
    nc.sync.dma_start(bo_sb[:], t_in["bo_v"].rearrange("(m p) -> p m", p=128))
    b1_sb = consts.tile([128, NF], F32)
    nc.sync.dma_start(b1_sb[:], t_in["b1_v"].rearrange("(f p) -> p f", p=128))
    b2b = consts.tile([128, D], F32)
    b2_d = t_in["b2_v"]
    b2_bc = bass.AP(tensor=b2_d.tensor, offset=b2_d.offset,
                    ap=[[0, 128]] + list(b2_d.ap))
    nc.sync.dma_start(b2b[:], b2_bc)
    if has_mask:
        mb_sb = consts.tile([128, NT], F32)
        nc.sync.dma_start(mb_sb[:], t_in["mb_v"].rearrange("(i p) -> p i", p=128))

    # ---------------- long-lived SBUF pools (left stack) ----------------
    poolA = tc.alloc_tile_pool(name="poolA", bufs=1)        # ln1T
    ln1T = poolA.tile([128, NR * S], BF16)                  # chunk r at [r*S, +S)
    poolWv = tc.alloc_tile_pool(name="poolWv", bufs=1)
    wv_sb = poolWv.tile([128, NR * D], BF16)
    # right stack: qk weights (dead after B), LN1 stream
    poolWqk = tc.alloc_tile_pool(name="poolWqk", bufs=1, side="right")
    wq_sb = poolWqk.tile([128, NR * D], BF16)
    wk_sb = poolWqk.tile([128, NR * D], BF16)

    psumAB = tc.alloc_tile_pool(name="psumAB", bufs=1, space="PSUM")

    # ------------- phases A+B interleaved: LN1/transpose + QKV ----------
    # DMA queue order: x0-3 | wq||x4-7 | wk||x8-11 | wv||x12-15 so the
    # projections can start while the tail of x is still arriving.
    poolKV = tc.alloc_tile_pool(name="poolKV", bufs=1)
    QT = poolKV.tile([128, NR * Q], BF16)                   # chunk m at [m*Q, +Q)
    KT = poolKV.tile([128, NR * S], BF16)                   # chunk m at [m*S, +S)
    V_sb = poolKV.tile([128, NT * H * VS], BF16)
    Vv = V_sb.rearrange("p (t h s) -> p t h s", t=NT, s=VS)

    streamA = tc.alloc_tile_pool(name="streamA", bufs=3, side="right")
    ln1Tv = ln1T.rearrange("p (r s) -> p r s", r=NR)

    def _ln1_iter(t):
        xt = streamA.tile([128, D], F32, bufs=3)
        nc.sync.dma_start(xt[:], x_all[t * 128:(t + 1) * 128, :])
        stats = streamA.tile([128, 2, 6], F32, bufs=4)
        xg = xt.rearrange("p (g d) -> p g d", g=2)
        nc.vector.bn_stats(stats[:, 0, :], xg[:, 0, :])
        nc.vector.bn_stats(stats[:, 1, :], xg[:, 1, :])
        mv = streamA.tile([128, 2], F32, bufs=4)
        nc.vector.bn_aggr(mv[:], stats[:])
        rstd = streamA.tile([128, 1], F32, bufs=4)
        nc.scalar.activation(rstd[:], mv[:, 1:2], AF.Sqrt, scale=float(D) / (D - 1))
        nc.vector.tensor_scalar_add(rstd[:], rstd[:], EPS)
        nc.vector.reciprocal(rstd[:], rstd[:])
        negmr = streamA.tile([128, 1], F32, bufs=4)
        nc.vector.tensor_scalar(
            out=negmr[:], in0=mv[:, 0:1], scalar1=rstd[:], scalar2=-1.0,
            op0=ALU.mult, op1=ALU.mult)
        lt = streamA.tile([128, D], BF16, bufs=3)
        nc.scalar.activation(lt[:], xt[:], AF.Identity, bias=negmr[:], scale=rstd[:])
        for rg in range(2):
            tp = psumAB.tile([128, 512], BF16, tag="tr", bufs=2)
            for k in range(4):
                r = rg * 4 + k
                nc.tensor.matmul(
                    tp[:, k * 128:(k + 1) * 128], lt[:, r * 128:(r + 1) * 128],
                    ident_bf[:], is_transpose=True, skip_group_check=True)
            dst = ln1Tv[:, rg * 4:(rg + 1) * 4, t * 128:(t + 1) * 128]
            src = tp.rearrange("p (k c) -> p k c", k=4)
            if rg == 0:
                nc.vector.tensor_copy(dst, src)
            else:
                nc.scalar.activation(dst, src, AF.Copy)

    def _k_proj(m, sp):
        kps = psumAB.tile([128, 1024], F32, tag="big", bufs=2)
        for half in range(2):
            s4 = sp * 2 + half
            for r in range(NR):
                nc.tensor.matmul(
                    kps[:, half * 512:(half + 1) * 512],
                    wk_sb[:, r * D + m * 128: r * D + (m + 1) * 128],
                    ln1T[:, r * S + s4 * Q: r * S + (s4 + 1) * Q],
                    start=(r == 0), stop=(r == NR - 1), skip_group_check=True)
        nc.vector.tensor_scalar_add(
            KT[:, m * S + sp * 1024: m * S + (sp + 1) * 1024], kps[:],
            bk_sb[:, m:m + 1])

    for t in range(4):
        _ln1_iter(t)
    for k in range(4):
        nc.sync.dma_start(wq_sb[:, 2 * k * D:(2 * k + 1) * D],
                          t_in["wq_b"][2 * k * 128:(2 * k + 1) * 128, :])
        nc.sync.dma_start(wq_sb[:, (2 * k + 1) * D:(2 * k + 2) * D],
                          t_in["wq_b"][(2 * k + 1) * 128:(2 * k + 2) * 128, :])
        _ln1_iter(4 + k)
    for m in range(NR):                            # Q (needs ln1T t0-3 + wq)
        qps = psumAB.tile([128, Q], F32, tag="mm512", bufs=2)
        for r in range(NR):
            nc.tensor.matmul(
                qps[:], wq_sb[:, r * D + m * 128: r * D + (m + 1) * 128],
                ln1T[:, r * S: r * S + Q], start=(r == 0), stop=(r == NR - 1))
        nc.vector.tensor_scalar_add(QT[:, m * Q:(m + 1) * Q], qps[:], bq_sb[:, m:m + 1])
    for k in range(4):
        nc.sync.dma_start(wk_sb[:, 2 * k * D:(2 * k + 1) * D],
                          t_in["wk_b"][2 * k * 128:(2 * k + 1) * 128, :])
        nc.sync.dma_start(wk_sb[:, (2 * k + 1) * D:(2 * k + 2) * D],
                          t_in["wk_b"][(2 * k + 1) * 128:(2 * k + 2) * 128, :])
        _ln1_iter(8 + k)
    for m in range(NR):                            # K tokens 0-1023
        _k_proj(m, 0)
    for k in range(4):
        nc.sync.dma_start(wv_sb[:, 2 * k * D:(2 * k + 1) * D],
                          t_in["wv_b"][2 * k * 128:(2 * k + 1) * 128, :])
        nc.sync.dma_start(wv_sb[:, (2 * k + 1) * D:(2 * k + 2) * D],
                          t_in["wv_b"][(2 * k + 1) * 128:(2 * k + 2) * 128, :])
        _ln1_iter(12 + k)
    streamA.release()
    if "B" not in PHASES:
        _dummy_out()
        poolWqk.release()
        psumAB.release()
        poolKV.release(); poolWv.release(); poolA.release()
        consts.release()
        return

    nc.vector.memset(Vv[:, :, :, 64:65], 1.0)     # ones column for row-sums
    for m in range(NR):                            # K tokens 1024-2047
        _k_proj(m, 1)
    for tp_ in range(NT // 2):                     # V heads 0-7 (s2=0)
        vps = psumAB.tile([128, 1024], F32, tag="big", bufs=2)
        for half in range(2):
            t = tp_ * 2 + half
            for r in range(NR):
                nc.tensor.matmul(
                    vps[:, half * 512:(half + 1) * 512],
                    ln1T[:, r * S + t * 128: r * S + (t + 1) * 128],
                    wv_sb[:, r * D: r * D + 512],
                    start=(r == 0), stop=(r == NR - 1), skip_group_check=True)
        for half in range(2):
            t = tp_ * 2 + half
            nc.vector.tensor_copy(
                Vv[:, t, 0:8, 0:64],
                vps[:, half * 512:(half + 1) * 512].rearrange("p (h d) -> p h d", d=64))
    poolWqk.release()
    if "C" not in PHASES:
        _dummy_out()
        psumAB.release()
        poolKV.release(); poolWv.release(); poolA.release()
        consts.release()
        return

    # ---------------- phase C: attention, pipelined per head pair -------
    poolE = tc.alloc_tile_pool(name="poolE", bufs=1)
    E0 = poolE.tile([128, NT * Q], BF16)
    E1 = poolE.tile([128, NT * Q], BF16)
    # prefetch x_own / wo into the space freed by wq/wk (right side)
    poolwo = tc.alloc_tile_pool(name="poolwo", bufs=1, side="right")
    wo_sb = poolwo.tile([128, NR * D], BF16)
    Ou = poolwo.tile([128, NR * Q], BF16)          # normalized attn out
    rec = poolwo.tile([128, Q], F32)               # per-head 1/rowsum rows 0,64
    poolX = tc.alloc_tile_pool(name="poolX", bufs=1, side="right")
    x_own = poolX.tile([128, NQ * D], F32)
    for r in range(NR):
        nc.sync.dma_start(wo_sb[:, r * D:(r + 1) * D], t_in["wo_b"][r * 128:(r + 1) * 128, :])
    for t in range(NQ):
        nc.sync.dma_start(x_own[:, t * D:(t + 1) * D], x_all[t * 128:(t + 1) * 128, :])
    nc.vector.memset(rec[:], 1.0)                  # rows 1-63,65-127 stay finite

    psumAB.release()
    psumC = tc.alloc_tile_pool(name="psumC", bufs=1, space="PSUM")

    def _emit_av(m, i, opsA, opsB):
        vs = i * H * VS
        nc.tensor.matmul(
            opsA[0:65, :], V_sb[:, vs + 2 * m * VS: vs + 2 * m * VS + 65],
            E0[:, i * Q:(i + 1) * Q], start=(i == 0), stop=(i == NT - 1),
            skip_group_check=True)
        nc.tensor.matmul(
            opsB[0:65, :], V_sb[:, vs + (2 * m + 1) * VS: vs + (2 * m + 1) * VS + 65],
            E1[:, i * Q:(i + 1) * Q], start=(i == 0), stop=(i == NT - 1),
            skip_group_check=True)

    # V heads 8-15 as 32 quarter-tasks (t, q): q=0 -> heads 8-11 (needed from
    # m=4), q=1 -> heads 12-15 (needed from m=6).  Spread into the PE slack
    # under the exp stream: 5/5/5/5/6/6 tasks for m=0..5.
    vtasks = [(t, q) for q in (0, 1) for t in range(NT)]
    VCNT = [5, 5, 5, 5, 6, 6, 0, 0]
    _voff = [sum(VCNT[:m]) for m in range(NR + 1)]

    def _vtask(t, q):
        vt = psumC.tile([128, 256], F32, tag="vq", bufs=1, name="vq")
        for r in range(NR):
            nc.tensor.matmul(
                vt[:], ln1T[:, r * S + t * 128: r * S + (t + 1) * 128],
                wv_sb[:, r * D + 512 + q * 256: r * D + 768 + q * 256],
                start=(r == 0), stop=(r == NR - 1))
        nc.vector.tensor_copy(
            Vv[:, t, 8 + 4 * q:12 + 4 * q, 0:64],
            vt.rearrange("p (h d) -> p h d", d=64))

    def _tail(m, opsA, opsB):
        # normalization: 1/rowsum broadcast via PE rank-1 matmul
        _emit_av(m, NT - 2, opsA, opsB)
        _emit_av(m, NT - 1, opsA, opsB)
        nc.vector.reciprocal(rec[0:1, :], opsA[64:65, :])
        nc.vector.reciprocal(rec[64:65, :], opsB[64:65, :])
        bc = psumC.tile([128, 512], F32, tag="bc", bufs=1, name="bc")
        nc.tensor.matmul(bc[:], ones2[0:65, :], rec[0:65, :],
                         start=True, stop=True)
        nc.vector.tensor_copy(Ou[0:64, m * Q:(m + 1) * Q], opsA[0:64, :])
        nc.vector.tensor_copy(Ou[64:128, m * Q:(m + 1) * Q], opsB[0:64, :])
        nc.vector.tensor_tensor(
            out=Ou[:, m * Q:(m + 1) * Q], in0=Ou[:, m * Q:(m + 1) * Q],
            in1=bc[:], op=ALU.mult)

    prev = None
    for m in range(NR):
        opsA = psumC.tile([128, Q], F32, tag="av", bufs=2, name="opsA")
        opsB = psumC.tile([128, Q], F32, tag="av", bufs=2, name="opsB")
        for ip in range(NT // 2):
            sA = psumC.tile([128, 1024], F32, tag="scA", bufs=1)
            sB = psumC.tile([128, 1024], F32, tag="scB", bufs=1)
            for half in range(2):
                i = 2 * ip + half
                ks = m * S + i * 128
                nc.tensor.matmul(
                    sA[:, half * 512:(half + 1) * 512], KT[0:64, ks:ks + 128],
                    QT[0:64, m * Q:(m + 1) * Q], start=True, stop=True,
                    skip_group_check=True)
                nc.tensor.matmul(
                    sB[:, half * 512:(half + 1) * 512], KT[64:128, ks:ks + 128],
                    QT[64:128, m * Q:(m + 1) * Q], start=True, stop=True,
                    skip_group_check=True)
            if has_mask:
                for half in range(2):
                    i = 2 * ip + half
                    nc.vector.tensor_scalar_add(
                        sA[:, half * 512:(half + 1) * 512],
                        sA[:, half * 512:(half + 1) * 512], mb_sb[:, i:i + 1])
                    nc.vector.tensor_scalar_add(
                        sB[:, half * 512:(half + 1) * 512],
                        sB[:, half * 512:(half + 1) * 512], mb_sb[:, i:i + 1])
            nc.scalar.activation(E0[:, ip * 1024:(ip + 1) * 1024], sA[:], AF.Exp)
            nc.scalar.activation(E1[:, ip * 1024:(ip + 1) * 1024], sB[:], AF.Exp)
            if ip == 0 and prev is not None:
                _tail(m - 1, *prev)          # previous pair's AV tail + normalize
            if ip > 0:
                _emit_av(m, 2 * (ip - 1), opsA, opsB)
                _emit_av(m, 2 * (ip - 1) + 1, opsA, opsB)
            cnt = VCNT[m]
            for ti in range(cnt):
                if ti * 8 // cnt == ip:
                    t_, q_ = vtasks[_voff[m] + ti]
                    _vtask(t_, q_)
        prev = (opsA, opsB)
    _tail(NR - 1, *prev)

    poolE.release()
    poolKV.release()
    poolWv.release()
    poolA.release()

    # FFN weight prefetch into the freed left space
    poolF = tc.alloc_tile_pool(name="poolF", bufs=1)
    w1_sb = poolF.tile([128, NR * DFF], BF16)      # din chunk r at [r*DFF, +DFF)
    H1T = poolF.tile([128, NF * Q], BF16)          # dff chunk f at [f*Q, +Q)
    for r in range(NR):
        nc.sync.dma_start(w1_sb[:, r * DFF:(r + 1) * DFF], t_in["w1_b"][r * 128:(r + 1) * 128, :])

    if "D" not in PHASES:
        _dummy_out()
        psumC.release()
        poolF.release()
        poolX.release(); poolwo.release()
        consts.release()
        return

    # ---------------- phase D: O-proj, residual, LN2 --------------------
    psumC.release()
    psumDE = tc.alloc_tile_pool(name="psumDE", bufs=1, space="PSUM")
    poolM = tc.alloc_tile_pool(name="poolM", bufs=1, side="right")
    mhaT = poolM.tile([128, NR * Q], F32)
    poolD2 = tc.alloc_tile_pool(name="poolD2", bufs=1)   # h, g token-major
    h_sb = poolD2.tile([128, NQ * D], F32)
    g_sb = poolD2.tile([128, NQ * D], BF16)
    gT = poolD2.tile([128, NR * Q], BF16)                # chunk r at [r*Q, +Q)
    gTv = gT.rearrange("p (r q) -> p r q", r=NR)

    def _o_proj(m):
        mps = psumDE.tile([128, Q], F32, tag="av", bufs=2, name="mps")
        for r in range(NR):
            nc.tensor.matmul(
                mps[:], wo_sb[:, r * D + m * 128: r * D + (m + 1) * 128],
                Ou[:, r * Q:(r + 1) * Q], start=(r == 0), stop=(r == NR - 1))
        nc.vector.tensor_scalar_add(mhaT[:, m * Q:(m + 1) * Q], mps[:], bo_sb[:, m:m + 1])

    def _h_block(t, rg):
        tp = psumDE.tile([128, 512], F32, tag="tr", bufs=2)
        for k in range(4):
            r = rg * 4 + k
            nc.tensor.matmul(
                tp[:, k * 128:(k + 1) * 128],
                mhaT[:, r * Q + t * 128: r * Q + (t + 1) * 128],
                ident_f[:], is_transpose=True, skip_group_check=True)
        nc.vector.tensor_tensor(
            out=h_sb[:, t * D + rg * 512: t * D + (rg + 1) * 512],
            in0=tp[:], in1=x_own[:, t * D + rg * 512: t * D + (rg + 1) * 512],
            op=ALU.add)

    for m in range(4):
        _o_proj(m)
    for t in range(NQ):
        _h_block(t, 0)
    for m in range(4, NR):
        _o_proj(m)
    for t in range(NQ):
        _h_block(t, 1)
    poolM.release()
    poolX.release()
    poolwo.release()

    # w2 prefetch now that the right stack is free
    poolF2 = tc.alloc_tile_pool(name="poolF2", bufs=1)
    w2_sb = poolF2.tile([128, NF * D], BF16)       # dff chunk f at [f*D, +D)
    for f in range(NF):
        nc.sync.dma_start(w2_sb[:, f * D:(f + 1) * D], t_in["w2_b"][f * 128:(f + 1) * 128, :])

    streamD = tc.alloc_tile_pool(name="streamD", bufs=4, side="right")
    for t in range(NQ):
        ht = h_sb[:, t * D:(t + 1) * D]
        stats2 = streamD.tile([128, 2, 6], F32, bufs=4)
        hg = ht.rearrange("p (g d) -> p g d", g=2)
        nc.vector.bn_stats(stats2[:, 0, :], hg[:, 0, :])
        nc.vector.bn_stats(stats2[:, 1, :], hg[:, 1, :])
        mv2 = streamD.tile([128, 2], F32, bufs=4)
        nc.vector.bn_aggr(mv2[:], stats2[:])
        rstd2 = streamD.tile([128, 1], F32, bufs=4)
        nc.scalar.activation(rstd2[:], mv2[:, 1:2], AF.Sqrt, scale=float(D) / (D - 1))
        nc.vector.tensor_scalar_add(rstd2[:], rstd2[:], EPS)
        nc.vector.reciprocal(rstd2[:], rstd2[:])
        negmr2 = streamD.tile([128, 1], F32, bufs=4)
        nc.vector.tensor_scalar(
            out=negmr2[:], in0=mv2[:, 0:1], scalar1=rstd2[:], scalar2=-1.0,
            op0=ALU.mult, op1=ALU.mult)
        nc.scalar.activation(g_sb[:, t * D:(t + 1) * D], ht, AF.Identity,
                             bias=negmr2[:], scale=rstd2[:])
    streamD.release()

    for t in range(NQ):
        for rg in range(2):
            tg = psumDE.tile([128, 512], BF16, tag="trb", bufs=2)
            for k in range(4):
                r = rg * 4 + k
                nc.tensor.matmul(
                    tg[:, k * 128:(k + 1) * 128],
                    g_sb[:, t * D + r * 128: t * D + (r + 1) * 128],
                    ident_bf[:], is_transpose=True, skip_group_check=True)
            dst = gTv[:, rg * 4:(rg + 1) * 4, t * 128:(t + 1) * 128]
            nc.vector.tensor_copy(dst, tg.rearrange("p (k c) -> p k c", k=4))
    # h2 = h + b2 (pre-added for the FFN2 residual)
    for t in range(NQ):
        nc.vector.tensor_tensor(
            out=h_sb[:, t * D:(t + 1) * D], in0=h_sb[:, t * D:(t + 1) * D],
            in1=b2b[:], op=ALU.add)
    if "E" not in PHASES:
        _dummy_out()
        psumDE.release()
        poolF2.release(); poolD2.release(); poolF.release()
        consts.release()
        return

    # ---------------- phase E: FFN --------------------------------------
    for f in range(NF):
        fps = psumDE.tile([128, Q], F32, tag="mm512", bufs=2)
        for r in range(NR):
            nc.tensor.matmul(
                fps[:], w1_sb[:, r * DFF + f * 128: r * DFF + (f + 1) * 128],
                gT[:, r * Q:(r + 1) * Q], start=(r == 0), stop=(r == NR - 1))
        nc.scalar.activation(
            H1T[:, f * Q:(f + 1) * Q], fps[:], AF.Gelu, bias=b1_sb[:, f:f + 1],
            scale=1.0)

    streamE = tc.alloc_tile_pool(name="streamE", bufs=2)
    for t in range(NQ):
        for s2 in range(2):
            ops2 = psumDE.tile([128, Q], F32, tag="av", bufs=2)
            for f in range(NF):
                nc.tensor.matmul(
                    ops2[:], H1T[:, f * Q + t * 128: f * Q + (t + 1) * 128],
                    w2_sb[:, f * D + s2 * Q: f * D + (s2 + 1) * Q],
                    start=(f == 0), stop=(f == NF - 1))
            ot = streamE.tile([128, Q], F32, bufs=2)
            nc.vector.tensor_tensor(
                out=ot[:], in0=ops2[:],
                in1=h_sb[:, t * D + s2 * Q: t * D + (s2 + 1) * Q], op=ALU.add)
            nc.sync.dma_start(out_d[t * 128:(t + 1) * 128, s2 * Q:(s2 + 1) * Q], ot[:])
    streamE.release()
    psumDE.release()
    poolF2.release()
    poolD2.release()
    poolF.release()
    consts.release()


_NC = {}


def _get_nc(has_mask=False):
    key = (has_mask, os.environ.get("BASS_REPEAT", "1"))
    if key not in _NC:
        _NC[key] = _build(has_mask)
    return _NC[key]


def _prep_in_maps(inputs):
    x = np.asarray(inputs["x"], np.float32)          # [2, 2048, 1024]
    mask = np.asarray(inputs["mask"])                # [2, 1, 1, 2048]
    wq, bq = np.asarray(inputs["wq"], np.float32), np.asarray(inputs["bq"], np.float32)
    wk, bk = np.asarray(inputs["wk"], np.float32), np.asarray(inputs["bk"], np.float32)
    wv, bv = np.asarray(inputs["wv"], np.float32), np.asarray(inputs["bv"], np.float32)
    wo, bo = np.asarray(inputs["wo"], np.float32), np.asarray(inputs["bo"], np.float32)
    ln1_w, ln1_b = np.asarray(inputs["ln1_w"], np.float32), np.asarray(inputs["ln1_b"], np.float32)
    ln2_w, ln2_b = np.asarray(inputs["ln2_w"], np.float32), np.asarray(inputs["ln2_b"], np.float32)
    w1, b1 = np.asarray(inputs["w1"], np.float32), np.asarray(inputs["b1"], np.float32)
    w2, b2 = np.asarray(inputs["w2"], np.float32), np.asarray(inputs["b2"], np.float32)

    has_mask = not bool(np.all(mask == 1))
    bf = ml_dtypes.bfloat16
    sc = 1.0 / np.sqrt(np.float32(DK))
    wq_b = (ln1_w[:, None] * wq * sc).astype(bf)
    wk_b = (ln1_w[:, None] * wk).astype(bf)
    wv_b = (ln1_w[:, None] * wv).astype(bf)
    wo_b = wo.astype(bf)
    w1_b = (ln2_w[:, None] * w1).astype(bf)
    w2_b = w2.astype(bf)
    bq_v = ((ln1_b @ wq + bq) * sc).astype(np.float32)
    bk_v = (ln1_b @ wk + bk).astype(np.float32)
    bv_full = ln1_b @ wv + bv
    bo_v = (bv_full @ wo + bo).astype(np.float32)
    b1_v = (ln2_b @ w1 + b1).astype(np.float32)
    b2_v = b2.astype(np.float32)

    common = dict(wq_b=wq_b, wk_b=wk_b, wv_b=wv_b, wo_b=wo_b, w1_b=w1_b,
                  w2_b=w2_b, bq_v=bq_v, bk_v=bk_v, bo_v=bo_v, b1_v=b1_v,
                  b2_v=b2_v)
    in_maps = []
    for c in range(NCHIP):
        b, j = c // 4, c % 4
        q0 = j * Q
        xr = np.concatenate([x[b, q0:], x[b, :q0]], axis=0)
        m = dict(common, x_all=np.ascontiguousarray(xr))
        if has_mask:
            mbias = np.where(mask[b, 0, 0] == 0, np.float32(-30000.0),
                             np.float32(0.0))
            m["mb_v"] = np.concatenate([mbias[q0:], mbias[:q0]]).astype(np.float32)
        in_maps.append(m)
    return in_maps, has_mask


def kernel(**inputs):
    in_maps, has_mask = _prep_in_maps(inputs)
    nc = _get_nc(has_mask)
    res = run_bass_kernel_spmd(nc, in_maps, core_ids=list(range(NCHIP)))
    out = np.empty((2, S, D), np.float32)
    for c in range(NCHIP):
        b, j = c // 4, c % 4
        out[b, j * Q:(j + 1) * Q] = res.results[c]["out"]
    return out
